# revision 2
# baseline (speedup 1.0000x reference)
"""Causal self-attention (B=2, L=2048, C=1024, 16 heads) on 8 Trainium2
NeuronCores via Bass/Tile — v2.

Sharding (8 cores = 2 batches x 4 head-groups of 4 heads):
  core c: batch b = c // 4, head-group g = c % 4  (heads 4g..4g+3)

v2 vs the baseline kernel:
  - q/k projections contract fp8e4 operands (weights pre-scaled x16 on the
    host so they sit in fp8's normal range), optionally with DoubleRow perf
    mode (virtual K=256, 2 MACs/cell/cycle).  Quantization noise cancels
    over the K=1024 contraction (~0.3%).  The softmax VALUE path (P, V, y)
    stays bf16: softmax outputs are averages whose magnitude shrinks with
    the attended count, so elementwise quantization noise does NOT cancel
    there (fp8 measured 6.4% rel err).
  - x is loaded once, in fp8 (2 MB); v_proj contracts fp8 x against fp8
    16*Wv at bf16 rate into a bf16 vaug.
  - scale chain: scores x256 -> exp scale 0.125/256; vaug holds 16v with a
    0.25 ones column so reciprocal(den row) = 4/den and yT = 64*y;
    out-proj psum = (64y)(16Wp) = 1024*out -> psum->sbuf copy multiplies
    by 2^-10.
  - the whole rep's attention is ONE software-pipelined stream over
    (chunk, head-pair, key-block-pair): scores for step s+1 are emitted
    before the P@V of step s, and projection / out-projection matmuls are
    popped from a deadline-ordered splice queue between them, so the
    strict-FIFO PE always has ready work while ScalarE runs exps (HAM
    stays warm).  Projections for token chunk tc execute during attention
    of chunk tc-1 (cross-rep for tc=0/1).
  - softmax denominator: DVE reciprocal of the PV ones-row, broadcast
    across 64 partitions either on the Pool engine (partition_broadcast)
    or via a K=1 PE matmul into the free upper psum rows (flag).
"""
import sys
import numpy as np
import ml_dtypes

for _p in ("/opt/trn_rl_repo",):
    if _p not in sys.path:
        sys.path.insert(0, _p)

import concourse.bass as bass
import concourse.mybir as mybir
import concourse.tile as tile
from concourse import bacc
from concourse import bass_utils

F32 = mybir.dt.float32
BF16 = mybir.dt.bfloat16
F8 = mybir.dt.float8e4
AF = mybir.ActivationFunctionType
DR = mybir.MatmulPerfMode.DoubleRow

N_CORES = 8
B, L, C, H, D = 2, 2048, 1024, 16, 64
H_PER_CORE = 4
DQ = H_PER_CORE * D          # 256 = per-core q/k/v width and out-column slice
NP = 4                       # channel pair-tiles (4 x 256 = C)
QCHUNK = 512
QC = L // QCHUNK             # 4 query chunks
NB = L // 128                # 16 key blocks
NEG = -30000.0
EXP_SCALE = 0.125 / 256.0    # 1/sqrt(D) / (16*16 weight scaling)


def build_kernel(use_collective=True, reps=1, use_dr_qk=False,
                 pool_ops=False, store_sync=True, io_f8=False):
    nc = bacc.Bacc("TRN2", target_bir_lowering=False, debug=False,
                   num_devices=N_CORES)

    IOT = F8 if io_f8 else BF16
    assert not (use_dr_qk and not io_f8), "DoubleRow needs fp8 operands"
    x2_d = nc.dram_tensor("x2", [C, L], IOT, kind="ExternalInput")
    wqkv2_d = nc.dram_tensor("wqkv2", [C, 3 * DQ], IOT, kind="ExternalInput")
    wp2_d = nc.dram_tensor("wp2", [DQ, C], IOT, kind="ExternalInput")
    kmask_d = nc.dram_tensor("kmask", [128, NB], F32, kind="ExternalInput")
    trimask_d = nc.dram_tensor("trimask", [128, 128], BF16,
                               kind="ExternalInput")
    out_ds = [nc.dram_tensor(f"out{q}", [128, C], BF16, kind="ExternalOutput")
              for q in range(QC)]

    store_eng = (lambda: nc.sync) if store_sync else (lambda: nc.gpsimd)

    with tile.TileContext(nc) as tc:
        import contextlib
        with contextlib.ExitStack() as ctx:
            const = ctx.enter_context(tc.tile_pool(name="const", bufs=1))
            kmask = const.tile([128, NB], F32)
            trimask = const.tile([128, 128], BF16)
            ones1 = const.tile([1, 64], BF16)

            sb = ctx.enter_context(tc.tile_pool(name="sb", bufs=1))
            pt_pool = ctx.enter_context(tc.tile_pool(name="pt", bufs=4))
            den_pool = ctx.enter_context(tc.tile_pool(name="den", bufs=3))
            stg = ctx.enter_context(tc.tile_pool(name="stg", bufs=3))
            ps = ctx.enter_context(tc.tile_pool(name="ps", bufs=1, space="PSUM"))

            x2t = [sb.tile([128, 2, L], IOT, tag=f"x2{i}", name=f"x2{i}")
                   for i in range(NP)]
            wqkv2t = [sb.tile([128, 2, 3 * DQ], IOT, tag=f"wq2{i}",
                              name=f"wq2{i}") for i in range(NP)]
            wp2t = sb.tile([128, 2, C], IOT, tag="wp2", name="wp2")
            qT = [[sb.tile([128, QCHUNK], BF16, tag=f"qT{qc}_{ht}",
                           name=f"qT{qc}_{ht}") for ht in range(2)]
                  for qc in range(QC)]
            kT = [[sb.tile([128, QCHUNK], BF16, tag=f"kT{tc}_{ht}",
                           name=f"kT{tc}_{ht}") for ht in range(2)]
                  for tc in range(QC)]
            # vaug2[jp]: key-block pair jp, [keys, half, head, dim+1];
            # column D holds 0.25 so the PV den row is 0.25*sum(p); 16*v
            vaug2 = [sb.tile([128, 2, H_PER_CORE, D + 1], BF16,
                             tag=f"va{jp}", name=f"va{jp}")
                     for jp in range(NB // 2)]
            # yT2[qc]: [chan-in-half, half(=ht), query] bf16, holds 64*y
            yT2 = [sb.tile([128, 2, QCHUNK], BF16, tag=f"yT{qc}",
                           name=f"yT{qc}") for qc in range(QC)]
            yR = [sb.tile([128, C], BF16, tag=f"yR{q}", name=f"yR{q}")
                  for q in range(QC)]
            dram = ctx.enter_context(tc.tile_pool(name="dram", bufs=1,
                                                  space="DRAM"))
            rs_ins = [dram.tile([QCHUNK, C], BF16, tag=f"rsi{q}",
                                name=f"rsi{q}") for q in range(QC)]
            rs_outs = [dram.tile([128, C], BF16, tag=f"rso{q}",
                                 name=f"rso{q}") for q in range(QC)]

            x2_view = x2_d.rearrange("(i j p) t -> i p j t", j=2, p=128)
            wq_view = wqkv2_d.rearrange("(i j p) m -> i p j m", j=2, p=128)
            wp_view = wp2_d.rearrange("(j p) c -> p j c", j=2)

            def load_dmas():
                for i in range(NP):
                    nc.sync.dma_start(out=x2t[i][:], in_=x2_view[i])
                for i in range(NP):
                    nc.sync.dma_start(out=wqkv2t[i][:], in_=wq_view[i])
                nc.sync.dma_start(out=wp2t[:], in_=wp_view)
                nc.sync.dma_start(out=kmask[:], in_=kmask_d[:])
                nc.sync.dma_start(out=trimask[:], in_=trimask_d[:])

            # ---- splice queue: projection work interleaved into attention
            SPLICE = []  # entries: (kind, fn); kind in {"proj", "oproj"}

            def pop_splice(n=1):
                # "proj" entries have emission deadlines (next chunk's
                # scores); pop them before deadline-free "oproj" entries
                while n > 0 and SPLICE:
                    for i, (k, _fn) in enumerate(SPLICE):
                        if k == "proj":
                            SPLICE.pop(i)[1]()
                            break
                    else:
                        SPLICE.pop(0)[1]()
                    n -= 1

            def flush_kind(kind):
                keep = []
                for k, fn in SPLICE:
                    if k == kind:
                        fn()
                    else:
                        keep.append((k, fn))
                SPLICE[:] = keep

            def flush_splice():
                while SPLICE:
                    SPLICE.pop(0)[1]()

            def qk_thunks(tc, off, dst):
                # q or k projection for tokens [512tc, 512tc+512), both ht
                sl = bass.ts(tc, QCHUNK)
                out = []
                for ht in range(2):
                    def t(ht=ht):
                        p = ps.tile([128, QCHUNK], F32, tag="psproj",
                                    name="psproj", bufs=2)
                        if use_dr_qk:
                            for i in range(NP):
                                nc.tensor.matmul(
                                    p[:],
                                    wqkv2t[i][:, :,
                                              off + ht*128:off + (ht+1)*128],
                                    x2t[i][:, :, sl],
                                    start=(i == 0), stop=(i == NP - 1),
                                    perf_mode=DR)
                        else:
                            for i in range(2 * NP):
                                nc.tensor.matmul(
                                    p[:],
                                    wqkv2t[i // 2][:, i % 2,
                                                   off + ht*128:
                                                   off + (ht+1)*128],
                                    x2t[i // 2][:, i % 2, sl],
                                    start=(i == 0), stop=(i == 2*NP - 1))
                        nc.vector.tensor_copy(dst[tc][ht][:], p[:])
                    out.append(("proj", t))
                return out

            def v_thunks(tc, first_rep):
                # v projection, one thunk per 128-token block (fp8 operands
                # at bf16 rate; psum = x^T @ 16Wv = 16v, stored bf16)
                out = []
                for tb in range(4):
                    tg = 4 * tc + tb
                    def t(tg=tg):
                        p = ps.tile([128, DQ], F32, tag="psproj",
                                    name="psproj", bufs=2)
                        for i in range(2 * NP):
                            nc.tensor.matmul(
                                p[:],
                                x2t[i // 2][:, i % 2, tg*128:(tg+1)*128],
                                wqkv2t[i // 2][:, i % 2, 2*DQ:3*DQ],
                                start=(i == 0), stop=(i == 2 * NP - 1))
                        jp, jj = tg // 2, tg % 2
                        if first_rep:
                            nc.vector.memset(vaug2[jp][:, jj, :, D:D+1], 0.25)
                        nc.vector.tensor_copy(
                            vaug2[jp][:, jj, :, 0:D],
                            p.rearrange("p (h d) -> p h d", h=H_PER_CORE))
                    out.append(("proj", t))
                return out

            def push_proj(tc, first_rep=False):
                SPLICE.extend(qk_thunks(tc, DQ, kT))
                SPLICE.extend(v_thunks(tc, first_rep))
                SPLICE.extend(qk_thunks(tc, 0, qT))

            def out_proj_thunks(qc):
                # partial out-proj: rs_in[mt rows] = (64y @ 16Wp) * 2^-10,
                # then ReduceScatter(add) within the 4-core batch group
                out = []
                for mt in range(4):
                    def t(mt=mt):
                        st = stg.tile([128, C], BF16, tag="st")
                        for half in range(2):
                            p = ps.tile([128, 512], F32, tag="psproj",
                                        name="psproj", bufs=2)
                            for m in range(2):
                                nc.tensor.matmul(
                                    p[:],
                                    yT2[qc][:, m, mt*128:(mt+1)*128],
                                    wp2t[:, m, half*512:(half+1)*512],
                                    start=(m == 0), stop=(m == 1))
                            nc.vector.tensor_scalar(
                                out=st[:, half*512:(half+1)*512], in0=p[:],
                                scalar1=2.0**-10, scalar2=None,
                                op0=mybir.AluOpType.mult)
                        store_eng().dma_start(
                            out=rs_ins[qc][mt*128:(mt+1)*128, :], in_=st[:])
                    out.append(("oproj", t))

                def fin():
                    if use_collective:
                        nc.gpsimd.collective_compute(
                            "ReduceScatter", mybir.AluOpType.add,
                            ins=[rs_ins[qc][:]], outs=[rs_outs[qc][:]],
                            replica_groups=[[0, 1, 2, 3], [4, 5, 6, 7]])
                        nc.sync.dma_start(out=yR[qc][:], in_=rs_outs[qc][:])
                        nc.sync.dma_start(out=out_ds[qc][:], in_=yR[qc][:])
                    else:
                        nc.sync.dma_start(out=out_ds[qc][:],
                                          in_=rs_ins[qc][0:128, :])
                out.append(("oproj", fin))
                return out

            # ---- attention: one software-pipelined stream per rep over
            # (qc, ht, key-block pair)
            def npair(qc):
                return (qc * QCHUNK + QCHUNK) // 256

            def emit_SE(qc, ht, jp):
                # scores + exp for both blocks of pair jp -> pt tile
                # pt layout [keys, pair-half jj, hp, query]
                q0 = qc * QCHUNK
                pt = pt_pool.tile([128, 2, 2, QCHUNK], BF16, tag="pt",
                                  name="pt")
                c0 = max(0, (2 * jp) * 128 - q0)
                for jj in (0, 1):
                    j = 2 * jp + jj
                    c_lo = max(0, j * 128 - q0)
                    pss = ps.tile([128, 2, QCHUNK], F32, tag="pss",
                                  name="pss", bufs=2)
                    for hp in (0, 64):
                        nc.tensor.matmul(
                            pss[:, hp // 64, c_lo:QCHUNK],
                            kT[j // 4][ht][hp:hp+64,
                                           (j % 4)*128:(j % 4 + 1)*128],
                            qT[qc][ht][hp:hp+64,
                                       bass.ds(c_lo, QCHUNK - c_lo)],
                            start=True, stop=True)
                    nc.scalar.activation(
                        pt[:, jj, :, c_lo:QCHUNK],
                        pss[:, :, c_lo:QCHUNK],
                        AF.Exp, bias=kmask[:, j:j+1], scale=EXP_SCALE)
                    if jj == 1 and c_lo > c0:
                        # zero the columns block j never computes but the
                        # P@V matmul streams
                        nc.vector.memset(pt[:, 1, :, c0:c_lo], 0.0)
                    if j * 128 >= q0:
                        dc = j * 128 - q0
                        eng = nc.gpsimd if pool_ops else nc.vector
                        eng.tensor_mul(
                            pt[:, jj, :, dc:dc+128],
                            pt[:, jj, :, dc:dc+128],
                            trimask[:, None, :].broadcast_to([128, 2, 128]))
                return pt

            def emit_PV(qc, ht, jp, pt, psys):
                q0 = qc * QCHUNK
                for jj in (0, 1):
                    j = 2 * jp + jj
                    c_lo = max(0, j * 128 - q0)
                    for hp_i, hp in enumerate((0, 64)):
                        h = 2 * ht + hp_i
                        nc.tensor.matmul(
                            psys[:D+1, hp_i, c_lo:QCHUNK],
                            vaug2[jp][:, jj, h, :],
                            pt[:, jj, hp_i, c_lo:QCHUNK],
                            start=(jp == 0 and jj == 0),
                            stop=(jp == npair(qc) - 1 and jj == 1))

            def emit_tail(qc, ht, psys):
                # yT = 64*y = psy * (4/den); den row = 0.25*sum(p).
                rden = den_pool.tile([1, 2, QCHUNK], BF16, tag="rden")
                with nc.allow_low_precision(
                        reason="1/den in bf16 feeds bf16 y"):
                    nc.vector.reciprocal(rden[:], psys[D:D+1, :, :])
                    if pool_ops:
                        rdb = den_pool.tile([64, 2, QCHUNK], BF16, tag="rdb")
                        nc.gpsimd.partition_broadcast(rdb[:], rden[:])
                        for hp_i, hp in enumerate((0, 64)):
                            nc.vector.tensor_mul(
                                yT2[qc][hp:hp+64, ht, :],
                                psys[0:D, hp_i, :],
                                rdb[:, hp_i, :])
                    else:
                        # broadcast 1/den across 64 partitions with a K=1
                        # matmul into the unused upper psum rows, stage via
                        # SBUF (DVE may read only one PSUM operand)
                        for hp_i, hp in enumerate((0, 64)):
                            nc.tensor.matmul(psys[64:128, hp_i, :], ones1[:],
                                             rden[:, hp_i, :],
                                             start=True, stop=True)
                            rdb = den_pool.tile([64, QCHUNK], BF16,
                                                tag="rdb")
                            nc.vector.tensor_copy(rdb[:],
                                                  psys[64:128, hp_i, :])
                            nc.vector.tensor_mul(
                                yT2[qc][hp:hp+64, ht, :],
                                psys[0:D, hp_i, :],
                                rdb[:])

            def attention_rep(r):
                seq = [(qc, ht, jp) for qc in range(QC) for ht in range(2)
                       for jp in range(npair(qc))]
                pts = {}
                psys = {}

                def alloc_psys(qc, ht):
                    psys[(qc, ht)] = ps.tile([128, 2, QCHUNK], F32,
                                             tag="psy", name="psy", bufs=1)

                def se(idx):
                    qc, ht, jp = seq[idx]
                    if jp == 0 and ht == 0:
                        if qc > 0:
                            # chunk crossing: projections for this chunk must
                            # precede these scores in the PE stream
                            flush_kind("proj")
                            if qc == 3:
                                if r + 1 < reps:
                                    load_dmas()
                                    push_proj(0)
                                    push_proj(1)
                            elif qc + 1 <= 3:
                                push_proj(qc + 1, first_rep=(r == 0))
                        elif r > 0:
                            flush_kind("proj")
                    if jp == 0:
                        alloc_psys(qc, ht)
                    pts[seq[idx]] = emit_SE(qc, ht, jp)

                se(0)
                for idx, (qc, ht, jp) in enumerate(seq):
                    if idx + 1 < len(seq):
                        se(idx + 1)
                    pop_splice(1)
                    emit_PV(qc, ht, jp, pts.pop((qc, ht, jp)),
                            psys[(qc, ht)])
                    pop_splice(1)
                    if jp == npair(qc) - 1:
                        emit_tail(qc, ht, psys.pop((qc, ht)))
                        # the next (ht/chunk) PV head-blocks on this tail's
                        # psys WAR; queue extra ready PE work ahead of it
                        pop_splice(4)
                        if ht == 1:
                            SPLICE.extend(out_proj_thunks(qc))

            for r in range(reps):
                if r == 0:
                    load_dmas()
                    nc.vector.memset(ones1[:], 1.0)
                    for _k, t in (qk_thunks(0, DQ, kT) + v_thunks(0, True)
                                  + qk_thunks(0, 0, qT)):
                        t()
                    push_proj(1, first_rep=True)
                attention_rep(r)
            flush_splice()

    nc.compile()
    return nc


def host_inputs(x, attn_mask, Wq, Wk, Wv, Wp, io_f8=False):
    x = np.asarray(x)
    attn_mask = np.asarray(attn_mask)
    Wq, Wk, Wv, Wp = (np.asarray(a) for a in (Wq, Wk, Wv, Wp))

    def f8(a):
        a = np.asarray(a, dtype=np.float32)
        if io_f8:
            return np.ascontiguousarray(
                np.clip(a, -240.0, 240.0)).astype(ml_dtypes.float8_e4m3)
        return np.ascontiguousarray(a).astype(ml_dtypes.bfloat16)

    tri = np.triu(np.ones((128, 128), np.float32)).astype(ml_dtypes.bfloat16)
    in_maps = []
    for c in range(N_CORES):
        b, g = c // 4, c % 4
        sl = slice(DQ * g, DQ * (g + 1))
        km = np.where(attn_mask[b] != 0, 0.0, NEG).astype(np.float32)
        km = np.ascontiguousarray(km.reshape(NB, 128).T)
        in_maps.append({
            "x2": f8(x[b].T),
            "wqkv2": f8(16.0 * np.concatenate(
                [Wq[sl, :].T, Wk[sl, :].T, Wv[sl, :].T], axis=1)),
            "wp2": f8(16.0 * Wp[:, sl].T),
            "kmask": km,
            "trimask": np.ascontiguousarray(tri),
        })
    return in_maps


_CACHED = {}


def kernel(x, attn_mask, Wq, Wk, Wv, Wp):
    if "nc" not in _CACHED:
        _CACHED["nc"] = build_kernel()
    nc = _CACHED["nc"]
    in_maps = host_inputs(x, attn_mask, Wq, Wk, Wv, Wp)
    res = bass_utils.run_bass_kernel_spmd(
        nc, in_maps, core_ids=list(range(N_CORES)))
    out = np.zeros((B, L, C), np.float32)
    for b in range(B):
        arr = np.stack(
            [np.stack([np.asarray(res.results[4*b + g][f"out{qc}"],
                                  dtype=np.float32) for qc in range(QC)])
             for g in range(4)], axis=1)
        out[b] = arr.reshape(L, C)
    return out
